# revision 1
# baseline (speedup 1.0000x reference)
"""Braid causal self-attention Trainium2 kernel (8-core SPMD).

Sharding: data-parallel over batch (2) x tensor-parallel over head groups (4).
Core c handles batch b=c//4, q-heads [4g:4g+4], kv-heads [2g:2g+2], g=c%4.
Each core computes a partial projection output (Wproj input-dim shard);
partials are summed on the host.

Key algebraic restructurings (validated vs reference in fp64):
  - q/k are only needed through the braid scores s_q/s_k. With
    g[d,t] = braid/rotary-folded weights and msq[d,t] = cos^2+sin^2,
    s = (sum_d q[d,t]*g[d,t]) * rsqrt(mean_d q[d,t]^2*msq[d,t] + eps),
    so rotary+rmsnorm are never materialized.
  - attn = sigmoid(s_k[j] + s_q[i]) is generated per 128-key block as a
    K=2 matmul from [2,N] slices of a score vector tile (value row + ones
    row), evaluated by the scalar engine's sigmoid, causally masked with a
    triangular multiply, and contracted with v via fp32r matmuls.
"""
import numpy as np
from contextlib import ExitStack

import concourse.bass as bass
import concourse.mybir as mybir
import concourse.tile as tile
from concourse import bacc
from concourse.bass_utils import run_bass_kernel_spmd

F32 = mybir.dt.float32
F32R = mybir.dt.float32r
AF = mybir.ActivationFunctionType

T = 2048
C = 1024
D = 64
EPS = 1e-6
NCORES = 8


def build_program():
    nc = bacc.Bacc()
    dp = nc.declare_dram_parameter
    xT_d = dp("xT", [C, T], F32, isOutput=False)          # x[b].T
    wq_d = dp("wq", [C, 256], F32, isOutput=False)        # Wq[group].T
    wk_d = dp("wk", [C, 128], F32, isOutput=False)
    wv_d = dp("wv", [C, 128], F32, isOutput=False)
    wp_d = dp("wp", [256, C], F32, isOutput=False)        # Wproj[:, group].T (prescaled)
    gm_d = dp("gm", [128, T], F32, isOutput=False)        # braid g (2-head dup)
    mh_d = dp("mh", [128, T], F32, isOutput=False)        # sqrt(cos^2+sin^2) (2-head dup)
    sel_d = dp("sel", [128, 3, 6], F32, isOutput=False)   # head selector masks
    trz_d = dp("trz", [128, 512], F32, isOutput=False)    # [zeros(384) | tri(i>=j)]
    idn_d = dp("idn", [128, 128], F32, isOutput=False)
    ones_d = dp("ones", [1, T], F32, isOutput=False)
    onz_d = dp("onz", [2, 128], F32, isOutput=False)      # [zeros; ones] K=2 bcast pair
    out_d = dp("outp", [T, C], F32, isOutput=True)
    out1_d = dp("outp1", [T, C], F32, isOutput=True)

    with tile.TileContext(nc) as tc, \
         nc.allow_low_precision("float32r output tags are bit-identical fp32"), \
         ExitStack() as ctx:
        cons = ctx.enter_context(tc.tile_pool(name="cons", bufs=1))
        work = ctx.enter_context(tc.tile_pool(name="work", bufs=1))

        # ---- constants / weights in SBUF ----
        wq_s = cons.tile([128, 8, 256], F32R)
        wk_s = cons.tile([128, 8, 128], F32R)
        wv_s = cons.tile([128, 8, 128], F32R)
        wp_s = cons.tile([128, 2, C], F32R)
        sel_s = cons.tile([128, 3, 6], F32R)
        trz_s = cons.tile([128, 512], F32R)
        idn_s = cons.tile([128, 128], F32)
        eps_t = cons.tile([128, 1], F32)
        nc.sync.dma_start(out=wq_s[:], in_=wq_d.ap().rearrange("(kt p) m -> p kt m", p=128).bitcast(F32R))
        nc.sync.dma_start(out=wk_s[:], in_=wk_d.ap().rearrange("(kt p) m -> p kt m", p=128).bitcast(F32R))
        nc.sync.dma_start(out=wv_s[:], in_=wv_d.ap().rearrange("(kt p) m -> p kt m", p=128).bitcast(F32R))
        nc.sync.dma_start(out=wp_s[:], in_=wp_d.ap().rearrange("(kt p) m -> p kt m", p=128).bitcast(F32R))
        nc.sync.dma_start(out=sel_s[:], in_=sel_d.ap().bitcast(F32R))
        nc.sync.dma_start(out=trz_s[:], in_=trz_d.ap().bitcast(F32R))
        nc.sync.dma_start(out=idn_s[:], in_=idn_d.ap())
        nc.vector.memset(eps_t[:], EPS)

        # long-lived work tiles
        vT = work.tile([128, T], F32)
        v_td = work.tile([128, T], F32R)  # 16 blocks of [t128, oc128]
        scomp = work.tile([6, T], F32)
        stil = work.tile([6, T], F32)     # s-tilde accumulator (pre-rsqrt)
        kcolT = work.tile([128, 2, 16], F32)   # s_k columns: [j, kh, jb]
        r1 = work.tile([6, T], F32)
        rq = work.tile([6, T], F32)
        yt0 = work.tile([128, T], F32R)  # heads 0,1 output (d-major)
        yt1 = work.tile([128, T], F32R)

        # ==== phase 1: projections with fused braid reductions ====
        # cn-major: for each 512-column chunk, project all four row-tiles
        # (q0, q1, k, v), then immediately compute the braid products from
        # PSUM and fold them into the per-chunk selector matmuls.
        with tc.tile_pool(name="bpool", bufs=2) as bp, \
             tc.tile_pool(name="xpool", bufs=1) as xp, \
             tc.tile_pool(name="pp1", bufs=2, space="PSUM") as pp1, \
             tc.tile_pool(name="pp2", bufs=2, space="PSUM") as pp2:
            gm_s = bp.tile([128, T], F32, tag="gm")
            mh_s = bp.tile([128, T], F32, tag="mh")
            nc.sync.dma_start(out=gm_s[:], in_=gm_d.ap())
            nc.sync.dma_start(out=mh_s[:], in_=mh_d.ap())
            xT_s = xp.tile([128, 8, T], F32R)
            nc.sync.dma_start(out=xT_s[:],
                              in_=xT_d.ap().rearrange("(kt p) t -> p kt t", p=128).bitcast(F32R))

            tiles = [(wq_s, 0, 0), (wq_s, 128, 1), (wk_s, 0, 2), (wv_s, 0, 3)]
            for cn in range(4):
                sl = slice(512 * cn, 512 * cn + 512)
                pss_t = pp2.tile([6, 512], F32, tag="pss")
                psq_t = pp2.tile([6, 512], F32, tag="psq")
                for w_s, oc0, t_i in tiles:
                    ps = pp1.tile([128, 512], F32, tag="pj")
                    for kt in range(8):
                        nc.tensor.matmul(
                            ps[:], w_s[:, kt, oc0:oc0 + 128],
                            xT_s[:, kt, sl],
                            start=(kt == 0), stop=(kt == 7))
                    if t_i == 3:
                        nc.vector.tensor_copy(vT[:, sl], ps[:])
                    else:
                        a_t = bp.tile([128, 512], F32R, tag="a")
                        b_t = bp.tile([128, 512], F32, tag="b")
                        b2_t = bp.tile([128, 512], F32R, tag="b2")
                        nc.vector.tensor_mul(a_t[:], ps[:], gm_s[:, sl])
                        nc.vector.tensor_mul(b_t[:], ps[:], mh_s[:, sl])
                        nc.vector.tensor_mul(b2_t[:], b_t[:], b_t[:])
                        nc.tensor.matmul(pss_t[:], sel_s[:, t_i, :], a_t[:],
                                         start=(t_i == 0), stop=(t_i == 2))
                        nc.tensor.matmul(psq_t[:], sel_s[:, t_i, :], b2_t[:],
                                         start=(t_i == 0), stop=(t_i == 2))
                # stash s-tilde and ln(ss/64 + eps) for this chunk
                nc.vector.tensor_copy(stil[:, sl], pss_t[:])
                nc.scalar.activation(r1[:, sl], psq_t[:], AF.Ln,
                                     bias=eps_t[0:6], scale=1.0 / 64.0)

            # v transposes: 16 x [128,128] -> v_td blocks
            for grp in range(4):
                ps_t = pp1.tile([128, 512], F32, tag="vtp")
                for k in range(4):
                    jb = 4 * grp + k
                    nc.tensor.transpose(
                        ps_t[:, 128 * k:128 * k + 128],
                        vT[:, 128 * jb:128 * jb + 128], idn_s[:])
                nc.vector.tensor_copy(v_td[:, 512 * grp:512 * grp + 512], ps_t[:])

        # rsqrt via exp(-0.5*ln(.)) and final braid scores
        nc.scalar.activation(rq[:], r1[:], AF.Exp, scale=-0.5)
        nc.vector.tensor_mul(scomp[:], stil[:], rq[:])

        # kcolT: s_k columns via a DRAM bounce (free transpose in the APs)
        ksc_d = nc.dram_tensor("kscratch", [2, T], F32)
        nc.sync.dma_start(out=ksc_d.ap(), in_=scomp[0:2, :])
        nc.sync.dma_start(
            out=kcolT[:],
            in_=ksc_d.ap().rearrange("r (b j) -> j r b", j=128))

        # ================= phase 3: attention =================
        with tc.tile_pool(name="svpool", bufs=1) as svp, \
             tc.tile_pool(name="atpool", bufs=6) as atp, \
             tc.tile_pool(name="pp3", bufs=2, space="PSUM") as pp3:
            # sv: blocks 0-1 = {sk_h, ones}; blocks 2-5 = {ones, sq_h};
            # zero-pair block at [6T, 6T+128) for the s_q broadcast matmul
            sv = svp.tile([2, 6 * T + 128], F32R)
            nc.sync.dma_start(out=sv[0:1, 0:2 * T], in_=scomp[0:2, :].bitcast(F32R))
            nc.sync.dma_start(out=sv[1:2, 0:2 * T],
                              in_=ones_d.ap().to_broadcast((2, T)).bitcast(F32R))
            nc.sync.dma_start(out=sv[0:1, 2 * T:6 * T],
                              in_=ones_d.ap().to_broadcast((4, T)).bitcast(F32R))
            nc.sync.dma_start(out=sv[1:2, 2 * T:6 * T], in_=scomp[2:6, :].bitcast(F32R))
            nc.sync.dma_start(out=sv[0:2, 6 * T:6 * T + 128], in_=onz_d.ap().bitcast(F32R))

            for h in range(4):
                kh = h // 2
                qbase = 2 * T + T * h
                for hs in (0, 1024):
                    y_ps = pp3.tile([64, 1024], F32, tag="yps")
                    # broadcast s_q over all partitions once per (h, half):
                    # out[j,i] = 0*1 + 1*s_q[i] via the [zeros; ones] pair
                    sqb = pp3.tile([128, 1024], F32, tag="sqb")
                    for off in (0, 512):
                        nc.tensor.matmul(
                            sqb[:, off:off + 512],
                            sv[0:2, 6 * T:6 * T + 128],
                            sv[0:2, qbase + hs + off:qbase + hs + off + 512],
                            start=True, stop=True)
                    jmax = (hs + 1024) // 128
                    # 512-aligned windows (f32r matmuls write full 512-wide
                    # psum bank windows; accumulation requires alignment)
                    first_w = [None] * 2
                    last_w = [None] * 2
                    spans = {}
                    for jb in range(jmax):
                        ws = (max(hs, 128 * jb) // 512) * 512
                        spans[jb] = ws
                        for ck in range((ws - hs) // 512, 2):
                            if first_w[ck] is None:
                                first_w[ck] = jb
                            last_w[ck] = jb
                    for jb in range(jmax):
                        ws = spans[jb]
                        W = hs + 1024 - ws
                        vstart = max(hs, 128 * jb)
                        at_t = atp.tile([128, 1024], F32R, tag="att")
                        # attn = sigmoid(s_q[i] + s_k[j]): s_k column as ACT bias.
                        # Only the causal width is computed; the [ws, vstart)
                        # strip holds stale-but-finite data that the mask zeroes.
                        nc.scalar.activation(at_t[:, vstart - ws:W],
                                             sqb[:, vstart - hs:1024],
                                             AF.Sigmoid, bias=kcolT[:, kh, jb:jb + 1])
                        # causal mask: zero the sub-diagonal strip and apply the
                        # triangular mask on the diagonal block in one multiply
                        # against [zeros(384) | tri]
                        strip = 128 * jb - ws
                        mw = strip + (128 if 128 * jb >= hs else 0)
                        if mw > 0:
                            nc.vector.tensor_mul(at_t[:, 0:mw], at_t[:, 0:mw],
                                                 trz_s[:, 384 - strip:384 - strip + mw])
                        # attn @ v accumulation (512-wide, bank-aligned)
                        for off in range(0, W, 512):
                            ck = (ws - hs + off) // 512
                            nc.tensor.matmul(
                                y_ps[:, ws - hs + off:ws - hs + off + 512],
                                v_td[:, 128 * jb + 64 * kh:128 * jb + 64 * kh + 64],
                                at_t[:, off:off + 512],
                                start=(first_w[ck] == jb), stop=(last_w[ck] == jb))
                    yt_dst = yt0 if h < 2 else yt1
                    r0 = 64 * (h % 2)
                    nc.vector.tensor_copy(yt_dst[r0:r0 + 64, hs:hs + 1024], y_ps[:])

        # ================= phase 4: output projection =================
        # split by K-half: the yt0 half is emitted right after heads 0-1
        # finish, overlapping with heads 2-3 attention; halves summed on host
        with tc.tile_pool(name="ostage", bufs=4) as osp, \
             tc.tile_pool(name="pp4", bufs=4, space="PSUM") as pp4:
            for k2, (yt_src, od) in enumerate([(yt0, out_d), (yt1, out1_d)]):
                for tt in range(16):
                    for cn in range(2):
                        ps_o = pp4.tile([128, 512], F32, tag="opj")
                        nc.tensor.matmul(ps_o[:],
                                         yt_src[:, 128 * tt:128 * tt + 128],
                                         wp_s[:, k2, 512 * cn:512 * cn + 512],
                                         start=True, stop=True)
                        o_t = osp.tile([128, 512], F32, tag="ost")
                        nc.vector.tensor_copy(o_t[:], ps_o[:])
                        nc.sync.dma_start(
                            out=od.ap()[128 * tt:128 * tt + 128, 512 * cn:512 * cn + 512],
                            in_=o_t[:])

    nc.compile()
    return nc


_PROGRAM = None


def _get_program():
    global _PROGRAM
    if _PROGRAM is None:
        _PROGRAM = build_program()
    return _PROGRAM


def _host_inputs(x, cos, sin, Wq, Wk, Wv, Wproj, w_braid):
    cos2 = cos[:, 0, :].astype(np.float32)   # [T, 32]
    sin2 = sin[:, 0, :].astype(np.float32)
    wb = w_braid.astype(np.float32)
    g64 = np.empty((64, T), np.float32)
    g64[:32] = wb[:32, None] * cos2.T - wb[32:, None] * sin2.T
    g64[32:] = wb[32:, None] * cos2.T + wb[:32, None] * sin2.T
    gm = np.concatenate([g64, g64], axis=0)
    mh1 = np.sqrt(cos2.T ** 2 + sin2.T ** 2).astype(np.float32)  # [32, T]
    mh64 = np.concatenate([mh1, mh1], axis=0)
    mh = np.concatenate([mh64, mh64], axis=0)

    sel = np.zeros((128, 3, 6), np.float32)
    sel[0:64, 0, 2] = 1.0
    sel[64:128, 0, 3] = 1.0
    sel[0:64, 1, 4] = 1.0
    sel[64:128, 1, 5] = 1.0
    sel[0:64, 2, 0] = 1.0
    sel[64:128, 2, 1] = 1.0

    tri = (np.arange(128)[None, :] >= np.arange(128)[:, None]).astype(np.float32)
    trz = np.concatenate([np.zeros((128, 384), np.float32), tri], axis=1)
    idn = np.eye(128, dtype=np.float32)
    ones = np.ones((1, T), np.float32)
    pscale = np.float32(1.0 / (T ** 0.5 + 1e-6))

    in_maps = []
    for c in range(NCORES):
        b, g = c // 4, c % 4
        in_maps.append({
            "xT": np.ascontiguousarray(x[b].T),
            "wq": np.ascontiguousarray(Wq[256 * g:256 * (g + 1)].T),
            "wk": np.ascontiguousarray(Wk[128 * g:128 * (g + 1)].T),
            "wv": np.ascontiguousarray(Wv[128 * g:128 * (g + 1)].T),
            "wp": np.ascontiguousarray((Wproj[:, 256 * g:256 * (g + 1)] * pscale).T),
            "gm": gm, "mh": mh, "sel": sel, "trz": trz, "idn": idn, "ones": ones,
            "onz": np.concatenate([np.zeros((1, 128), np.float32),
                                   np.ones((1, 128), np.float32)], axis=0),
        })
    return in_maps


def kernel(x, cos, sin, Wq, Wk, Wv, Wproj, w_braid):
    x = np.asarray(x, np.float32)
    nc = _get_program()
    in_maps = _host_inputs(np.asarray(x, np.float32), np.asarray(cos), np.asarray(sin),
                           np.asarray(Wq, np.float32), np.asarray(Wk, np.float32),
                           np.asarray(Wv, np.float32), np.asarray(Wproj, np.float32),
                           np.asarray(w_braid, np.float32))
    res = run_bass_kernel_spmd(nc, in_maps, list(range(NCORES)))
    out = np.zeros((2, T, C), np.float32)
    for c in range(NCORES):
        out[c // 4] += res.results[c]["outp"]
        out[c // 4] += res.results[c]["outp1"]
    return out



# revision 8
# speedup vs baseline: 1.3774x; 1.3774x over previous
"""Braid causal self-attention Trainium2 kernel (8-core SPMD), v2.

Sharding: data-parallel over batch (2) x tensor-parallel over head groups (4).
Core c handles batch b=c//4, q-heads [4g:4g+4], kv-heads [2g:2g+2], g=c%4.
Each core emits a bf16 partial projection (its 256 Wproj input dims);
partials are summed on the host in fp32.

v2 design (vs fp32r baseline):
  - bf16 matmul operands everywhere (same PE rate as fp32r at wide free
    dims, half the DMA/SBUF, no 512-wide fp32r psum-window constraint).
  - Per-chunk score pipeline: braid scores s_q/s_k (rotary+rmsnorm folded
    into gm/mh as in v1) are reduced, normalized (Ln/Exp rsqrt), bounced
    through DRAM and broadcast back per 512-column chunk, so attention can
    start as soon as projections finish.
  - Attention is jb-outer (key-block) over full-T query windows with exact
    causal widths. Sigmoid runs once per (kv-pair, jb) covering both
    q-heads of the pair in a single ACT instruction (bias = s_k column).
  - attn@v uses v as stationary [j,64] with the two heads of a pair
    col-tiled into psum partitions [0:64] / [64:128] (concurrent PE
    sub-array execution).
  - y psum banks are [128,512] query-groups; when key loop passes a group
    boundary the group is final: copied to bf16 yt, and its banks are
    immediately reused for the output projection of those query columns.
  - v transposed to [t,d] layout via DMA-transpose (no PE/psum cost).
"""
import numpy as np
from contextlib import ExitStack

import ml_dtypes

import concourse.bass as bass
import concourse.mybir as mybir
import concourse.tile as tile
from concourse import bacc
from concourse.bass_utils import run_bass_kernel_spmd

F32 = mybir.dt.float32
BF16 = mybir.dt.bfloat16
AF = mybir.ActivationFunctionType

T = 2048
C = 1024
D = 64
EPS = 1e-6
NCORES = 8
BF = ml_dtypes.bfloat16


def build_program():
    nc = bacc.Bacc()
    dp = nc.declare_dram_parameter
    xT_d = dp("xT", [C, T], BF16, isOutput=False)          # x[b].T
    wq_d = dp("wq", [C, 256], BF16, isOutput=False)        # Wq[group].T
    wk_d = dp("wk", [C, 128], BF16, isOutput=False)
    wv_d = dp("wv", [C, 128], BF16, isOutput=False)
    wp_d = dp("wp", [256, C], BF16, isOutput=False)        # Wproj[:, group].T (prescaled)
    gm_d = dp("gm", [128, T], F32, isOutput=False)         # braid g (2-head dup)
    mh_d = dp("mh", [128, T], F32, isOutput=False)         # sqrt(cos^2+sin^2) (2-head dup)
    sel_d = dp("sel", [128, 3, 34], BF16, isOutput=False)  # head selector masks
    tri2_d = dp("tri2", [128, 2, 128], BF16, isOutput=False)  # causal tri, 2-head dup
    out_d = dp("outp", [T, C], BF16, isOutput=True)

    ssk_d = nc.dram_tensor("ssk", [2, T], F32)    # s_k bounce (bias columns)
    ssq_d = nc.dram_tensor("ssq", [4, T], BF16)   # s_q bounce (broadcast rows)

    with tile.TileContext(nc) as tc, \
         nc.allow_low_precision("bf16 kernel; rel-err budget 2e-2"), \
         ExitStack() as ctx:
        cons = ctx.enter_context(tc.tile_pool(name="cons", bufs=1))
        work = ctx.enter_context(tc.tile_pool(name="work", bufs=1))

        # ---- constants / weights in SBUF ----
        xT_s = cons.tile([128, 8, T], BF16)
        wq_s = cons.tile([128, 8, 256], BF16)
        wk_s = cons.tile([128, 8, 128], BF16)
        wv_s = cons.tile([128, 8, 128], BF16)
        wp_s = cons.tile([128, 2, C], BF16)
        gm_s = cons.tile([128, T], F32)
        mh_s = cons.tile([128, T], F32)
        sel_s = cons.tile([128, 3, 34], BF16)
        tri2_s = cons.tile([128, 2, 128], BF16)
        eps_t = cons.tile([128, 1], F32)
        nc.sync.dma_start(out=xT_s[:], in_=xT_d.ap().rearrange("(kt p) t -> p kt t", p=128))
        nc.sync.dma_start(out=wq_s[:], in_=wq_d.ap().rearrange("(kt p) m -> p kt m", p=128))
        nc.sync.dma_start(out=wk_s[:], in_=wk_d.ap().rearrange("(kt p) m -> p kt m", p=128))
        nc.sync.dma_start(out=wv_s[:], in_=wv_d.ap().rearrange("(kt p) m -> p kt m", p=128))
        nc.sync.dma_start(out=wp_s[:], in_=wp_d.ap().rearrange("(kt p) m -> p kt m", p=128))
        nc.sync.dma_start(out=gm_s[:], in_=gm_d.ap())
        nc.sync.dma_start(out=mh_s[:], in_=mh_d.ap())
        nc.sync.dma_start(out=sel_s[:], in_=sel_d.ap())
        nc.sync.dma_start(out=tri2_s[:], in_=tri2_d.ap())
        nc.vector.memset(eps_t[:], EPS)

        # long-lived work tiles
        vb = work.tile([128, T], BF16)          # v (d-major) staging for transpose
        v_td = work.tile([128, 16, 128], BF16)  # v blocks [t128, d128]
        sqb = work.tile([128, 4, T], BF16)      # s_q broadcast per head
        kcolT = work.tile([128, 16, 2], F32)    # s_k columns: [j, jb, pair]
        yt0 = work.tile([128, T], BF16)         # pair 0 y (d-major)
        yt1 = work.tile([128, T], BF16)
        yts = (yt0, yt1)

        # ==== phase A: projections + braid score pipeline ====
        with tc.tile_pool(name="bp", bufs=2) as bp, \
             tc.tile_pool(name="ppA", bufs=2, space="PSUM") as ppA, \
             tc.tile_pool(name="ppS", bufs=2, space="PSUM") as ppS:
            for cn in range(4):
                sl = slice(512 * cn, 512 * cn + 512)
                pss = ppS.tile([34, 512], F32, tag="pss")
                psq = ppS.tile([34, 512], F32, tag="psq")
                for t_i, (w_s, oc0) in enumerate(
                        [(wq_s, 0), (wq_s, 128), (wk_s, 0)]):
                    ps = ppA.tile([128, 512], F32, tag="pj")
                    for kt in range(8):
                        nc.tensor.matmul(
                            ps[:], w_s[:, kt, oc0:oc0 + 128], xT_s[:, kt, sl],
                            start=(kt == 0), stop=(kt == 7))
                    a_t = bp.tile([128, 512], BF16, tag="a")
                    b_t = bp.tile([128, 512], BF16, tag="b")
                    b2_t = bp.tile([128, 512], BF16, tag="b2")
                    nc.vector.tensor_mul(a_t[:], ps[:], gm_s[:, sl])
                    nc.vector.tensor_mul(b_t[:], ps[:], mh_s[:, sl])
                    nc.vector.tensor_mul(b2_t[:], b_t[:], b_t[:])
                    nc.tensor.matmul(pss[:], sel_s[:, t_i, :], a_t[:],
                                     start=(t_i == 0), stop=(t_i == 2))
                    nc.tensor.matmul(psq[:], sel_s[:, t_i, :], b2_t[:],
                                     start=(t_i == 0), stop=(t_i == 2))
                # v projection for this chunk + transpose to [t, d]
                psv = ppA.tile([128, 512], F32, tag="pj")
                for kt in range(8):
                    nc.tensor.matmul(
                        psv[:], wv_s[:, kt, :], xT_s[:, kt, sl],
                        start=(kt == 0), stop=(kt == 7))
                nc.vector.tensor_copy(vb[:, sl], psv[:])
                for k in range(4):
                    tb = 4 * cn + k
                    nc.sync.dma_start(
                        out=v_td[:, tb, :],
                        in_=vb[:, 128 * tb:128 * tb + 128],
                        transpose=True)
                # braid scores for this chunk:
                # s = s~ * rsqrt(ssq/64 + eps), rsqrt via exp(-0.5*ln(.))
                r1c = bp.tile([34, 512], F32, tag="r1")
                rqc = bp.tile([34, 512], F32, tag="rq")
                scc = bp.tile([34, 512], F32, tag="scc")
                scb = bp.tile([4, 512], BF16, tag="scb")
                nc.scalar.activation(r1c[:], psq[:], AF.Ln,
                                     bias=eps_t[0:34], scale=1.0 / 64.0)
                nc.scalar.activation(rqc[:], r1c[:], AF.Exp, scale=-0.5)
                nc.vector.tensor_mul(scc[:], pss[:], rqc[:])
                nc.vector.tensor_copy(scb[:], scc[0:4, :])
                nc.sync.dma_start(out=ssk_d.ap()[:, sl], in_=scc[32:34, :])
                nc.sync.dma_start(out=ssq_d.ap()[:, sl], in_=scb[:])
                for r in (0, 1):
                    nc.sync.dma_start(
                        out=kcolT[:, 4 * cn:4 * cn + 4, r],
                        in_=ssk_d.ap()[r:r + 1, sl].rearrange("o (b j) -> (o j) b", j=128))
                for h in range(4):
                    nc.sync.dma_start(
                        out=sqb[:, h, sl],
                        in_=ssq_d.ap()[h:h + 1, sl].to_broadcast((128, 512)))

        # ==== phase B: attention + progressive output projection ====
        with tc.tile_pool(name="atp", bufs=3) as atp, \
             tc.tile_pool(name="osp", bufs=3) as osp, \
             tc.tile_pool(name="ppB", bufs=1, space="PSUM") as ppB:
            y = {}
            for p in (0, 1):
                for gq in range(4):
                    y[(p, gq)] = ppB.tile([128, 512], F32, name=f"y{p}{gq}", tag=f"y{p}{gq}")
            for jb in range(16):
                W = T - 128 * jb
                for p in (0, 1):
                    at_t = atp.tile([128, 2, T], BF16, tag=f"at{p}")
                    # attn = sigmoid(s_q[i] + s_k[j]) for both heads of the
                    # pair in one ACT call; s_k column as per-partition bias
                    nc.scalar.activation(
                        at_t[:, :, 0:W], sqb[:, 2 * p:2 * p + 2, 128 * jb:T],
                        AF.Sigmoid, bias=kcolT[:, jb, p:p + 1])
                    # causal mask on the diagonal block
                    nc.vector.tensor_mul(at_t[:, :, 0:128], at_t[:, :, 0:128],
                                         tri2_s[:])
                    for gq in range(jb // 4, 4):
                        w0 = max(128 * jb, 512 * gq)
                        wlen = 512 * gq + 512 - w0
                        for hh in (0, 1):
                            nc.tensor.matmul(
                                y[(p, gq)][64 * hh:64 * hh + 64,
                                           w0 - 512 * gq:w0 - 512 * gq + wlen],
                                v_td[:, jb, 64 * p:64 * p + 64],
                                at_t[:, hh, w0 - 128 * jb:w0 - 128 * jb + wlen],
                                start=(jb == 0), stop=(jb == 4 * gq + 3))
                if jb % 4 == 3:
                    # query group gq is final for both pairs: stash to bf16,
                    # then project those columns (psum banks recycled via tag)
                    gq = jb // 4
                    for p in (0, 1):
                        nc.vector.tensor_copy(
                            yts[p][:, 512 * gq:512 * gq + 512], y[(p, gq)][:])
                    for tt in range(4):
                        tcol = 512 * gq + 128 * tt
                        o_ps0 = ppB.tile([128, 512], F32, name=f"o0{gq}", tag=f"y0{gq}")
                        o_ps1 = ppB.tile([128, 512], F32, name=f"o1{gq}", tag=f"y1{gq}")
                        for cn2, o_ps in ((0, o_ps0), (1, o_ps1)):
                            nc.tensor.matmul(
                                o_ps[:], yt0[:, tcol:tcol + 128],
                                wp_s[:, 0, 512 * cn2:512 * cn2 + 512],
                                start=True, stop=False)
                            nc.tensor.matmul(
                                o_ps[:], yt1[:, tcol:tcol + 128],
                                wp_s[:, 1, 512 * cn2:512 * cn2 + 512],
                                start=False, stop=True)
                        ob = osp.tile([128, C], BF16, tag="ob")
                        nc.vector.tensor_copy(ob[:, 0:512], o_ps0[:])
                        nc.vector.tensor_copy(ob[:, 512:C], o_ps1[:])
                        nc.sync.dma_start(
                            out=out_d.ap()[tcol:tcol + 128, :], in_=ob[:])

    nc.compile()
    return nc


_PROGRAM = None


def _get_program():
    global _PROGRAM
    if _PROGRAM is None:
        _PROGRAM = build_program()
    return _PROGRAM


def _host_inputs(x, cos, sin, Wq, Wk, Wv, Wproj, w_braid):
    cos2 = cos[:, 0, :].astype(np.float32)   # [T, 32]
    sin2 = sin[:, 0, :].astype(np.float32)
    wb = w_braid.astype(np.float32)
    g64 = np.empty((64, T), np.float32)
    g64[:32] = wb[:32, None] * cos2.T - wb[32:, None] * sin2.T
    g64[32:] = wb[32:, None] * cos2.T + wb[:32, None] * sin2.T
    gm = np.concatenate([g64, g64], axis=0)
    mh1 = np.sqrt(cos2.T ** 2 + sin2.T ** 2).astype(np.float32)  # [32, T]
    mh64 = np.concatenate([mh1, mh1], axis=0)
    mh = np.concatenate([mh64, mh64], axis=0)

    # selector output rows: 0-3 = s_q heads 0-3, 32-33 = s_k kv heads
    # (32-aligned partition bases for engine access)
    sel = np.zeros((128, 3, 34), np.float32)
    sel[0:64, 0, 0] = 1.0
    sel[64:128, 0, 1] = 1.0
    sel[0:64, 1, 2] = 1.0
    sel[64:128, 1, 3] = 1.0
    sel[0:64, 2, 32] = 1.0
    sel[64:128, 2, 33] = 1.0

    tri = (np.arange(128)[None, :] >= np.arange(128)[:, None]).astype(np.float32)
    tri2 = np.stack([tri, tri], axis=1)  # [128, 2, 128]
    pscale = np.float32(1.0 / (T ** 0.5 + 1e-6))

    in_maps = []
    for c in range(NCORES):
        b, g = c // 4, c % 4
        in_maps.append({
            "xT": np.ascontiguousarray(x[b].T).astype(BF),
            "wq": np.ascontiguousarray(Wq[256 * g:256 * (g + 1)].T).astype(BF),
            "wk": np.ascontiguousarray(Wk[128 * g:128 * (g + 1)].T).astype(BF),
            "wv": np.ascontiguousarray(Wv[128 * g:128 * (g + 1)].T).astype(BF),
            "wp": np.ascontiguousarray(
                (Wproj[:, 256 * g:256 * (g + 1)] * pscale).T).astype(BF),
            "gm": gm, "mh": mh,
            "sel": sel.astype(BF), "tri2": tri2.astype(BF),
        })
    return in_maps


def kernel(x, cos, sin, Wq, Wk, Wv, Wproj, w_braid):
    nc = _get_program()
    in_maps = _host_inputs(np.asarray(x, np.float32), np.asarray(cos), np.asarray(sin),
                           np.asarray(Wq, np.float32), np.asarray(Wk, np.float32),
                           np.asarray(Wv, np.float32), np.asarray(Wproj, np.float32),
                           np.asarray(w_braid, np.float32))
    res = run_bass_kernel_spmd(nc, in_maps, list(range(NCORES)))
    out = np.zeros((2, T, C), np.float32)
    for c in range(NCORES):
        out[c // 4] += res.results[c]["outp"].astype(np.float32)
    return out
